# revision 37
# baseline (speedup 1.0000x reference)
"""Dense MoE layer (8 experts, all-expert weighted combine) on 8 TRN2 NeuronCores.

Strategy: data-parallel over the token dim. Each core gets a 1024-token shard
(pre-transposed + bf16-cast on host), the full stacked expert weights (bf16),
and computes gate softmax + all 8 expert matmuls + gate-weighted combine
locally. No collectives; host concatenates the 8 output shards.

The kernel is PE-streaming-bound: 1024 expert matmuls of [128x128]@[128x512]
at the 216 ns bf16 floor. The schedule minimizes everything around that
stream:

  - phase A (expert 0 + gate) runs CHUNK-major over tile-PAIRS so the PE
    starts on real work as soon as the first x / We[0] chunks land instead of
    waiting for the full critical prefix. x ships in half-token chunks: pair 0
    (tiles 0,1) + the first gate half need only xa + We[0] = 3MB before
    full-rate compute; xb streams in behind them and pair 2's steps carry the
    second gate half's matmuls. 12 dummy matmuls on a memset tile warm the
    HAM clock gate through the DMA lead-in, handing off seamlessly so the
    DMA-chased pair 0 runs at 2.4 GHz.
  - gate logits are computed TRANSPOSED (lhsT = Wg chunk, 8-column weight
    loads are ~free, padded to 128 cols) into [128,512] psum per half. exp(z+bg) goes straight from
    psum to [8,tok] SBUF tensors on ACT (f32 for the softmax transposes, bf16
    for the bias matmul) - the gate bias rides ACT's per-partition bias port,
    so there is no separate bias add and no logit copy. Per tile, softmax is:
    PE-transpose the f32 exp back to [128,8], one DVE copy-with-accum (the
    denominator), reciprocal, scale. The bias term exp(z+bg) @ be is deferred
    to the e=1 sweep (phase B) so phase A can hold THREE [128,1024] psum
    y-slots (6 banks) + 2 shared single-bank "g8" slots.
  - phase B (experts 1..7) is token-major with 16 matmuls per (e,t) block
    accumulating in psum f32; combine is one fused DVE op:
    out = psum*g[:,e] + out. Expert weights double-buffer through SBUF.
  - DMA queues move ~one 256KB chunk per 1.3us each, so the critical
    prefix is split need-ordered across both hardware queues: sync = xa,
    then We[1], then xb, then outputs; scalar = We[0], then per-expert
    prefetch; gpsimd = small consts. xb/We[1] are emitted after gate_post(0)
    and ride sync so their dma_start issues never sit in front of the
    pair-0-gated exps on scalar (that chain gates the first softmax
    transpose).
  - tail: the last (e,t) block computes its second half as two [128,256]
    psum groups in the (by then free) g8 banks, so only a 256-col combine +
    writeback trails the final matmul, on otherwise-idle engines.
  - experts 5..6 run in fp8 e4m3 with perf_mode=DoubleRow: the PE packs two
    fp8 weights per cell, so each [256x128]@[256x512] pair-matmul covers two
    128-chunks of the contraction in ~one matmul time (2x ALU; measured
    ~216ns/pair-MM = 1.79x per block). x ships a second copy quantized as
    e4m3(x/8) (stationary) and We[5..6] as e4m3(8*We) (moving); the two
    power-of-2 scales cancel exactly, so the combine path is unchanged.
    Quantization noise on 2 of 8 experts puts the end-to-end rel err at
    ~1.8e-2 vs the 2e-2 gate (bf16 path alone: 2.5e-3). Expert 7 stays bf16
    and runs LAST: its 27.6us sweep hides the ~21us of per-tile output
    writebacks that a 13.7us fp8 sweep cannot (measured: fp8-last stalls
    5.9us at the tail). The fp8 tensors stream mid-phase-B on the scalar
    queue (x8 at the e=3 sweep, W8[j] one sweep ahead of use), far off the
    critical prefix.
"""

import os
import sys

import numpy as np

try:
    import concourse.bass as bass  # noqa: F401
except ImportError:  # harness containers stage the repo at /opt/trn_rl_repo
    sys.path.insert(0, "/opt/trn_rl_repo")

from contextlib import ExitStack

import ml_dtypes

import concourse.bass as bass
import concourse.mybir as mybir
import concourse.tile as tile
from concourse import bacc
from concourse.bass_utils import run_bass_kernel_spmd

N_CORES = 8
N_TOK = 8192
IN_F = 1024
OUT_F = 1024
E = 8
P = 128  # partitions
N_FP8 = 2  # experts 5..6 run fp8/DoubleRow; 0..4 and 7 stay bf16
FP8_E0 = 5  # first fp8 expert
E_BF = E - N_FP8  # bf16 expert count (slots 0..4 = experts 0..4, slot 5 = expert 7)


def build_nc(n_tok_pc: int = N_TOK // N_CORES, debug: bool = False):
    """Build the single-core SPMD Bass program (same program on all 8 cores)."""
    fp32 = mybir.dt.float32
    bf16 = mybir.dt.bfloat16
    f8 = mybir.dt.float8e4

    K_CH = IN_F // P  # contraction chunks of 128
    T = n_tok_pc // P  # token tiles per core
    assert T >= 4 and T % 2 == 0

    nc = bacc.Bacc(
        "TRN2", target_bir_lowering=False, debug=debug, enable_asserts=False
    )

    # both token halves in SBUF layout [p, chunk, tok] so they ship in few
    # large DMAs (each dma_start costs ~600ns of engine issue time, and 16
    # per-chunk transfers rotate through too few DMA semaphores - the false
    # coupling made pair-0 matmuls wait on unrelated xb chunks)
    xTa = nc.declare_dram_parameter(
        "xTa", [P, IN_F // P, n_tok_pc // 2], bf16, isOutput=False
    )
    xTb = nc.declare_dram_parameter(
        "xTb", [P, IN_F // P, n_tok_pc // 2], bf16, isOutput=False
    )
    We = nc.declare_dram_parameter("We", [E_BF, IN_F, OUT_F], bf16, isOutput=False)
    # fp8 copies for the DoubleRow experts: x8[p, c, n] = e4m3(x[n, c*128+p]/8),
    # W8[j, p, cp, i, o] = e4m3(8*We[FP8_E0+j, (2cp+i)*128+p, o])
    x8d = nc.declare_dram_parameter("x8", [P, K_CH, n_tok_pc], f8, isOutput=False)
    W8d = nc.declare_dram_parameter(
        "W8", [N_FP8, P, K_CH // 2, 2, OUT_F], f8, isOutput=False
    )
    be = nc.declare_dram_parameter("be", [E, OUT_F], bf16, isOutput=False)
    Wg = nc.declare_dram_parameter("Wg", [P, K_CH, P], bf16, isOutput=False)
    bgc = nc.declare_dram_parameter("bgc", [E, 1], fp32, isOutput=False)
    idn = nc.declare_dram_parameter("idn", [P, P], fp32, isOutput=False)
    out = nc.declare_dram_parameter("out", [n_tok_pc, OUT_F], fp32, isOutput=True)

    with tile.TileContext(nc) as tc, ExitStack() as ctx:
        consts = ctx.enter_context(tc.tile_pool(name="consts", bufs=1))
        xpool = ctx.enter_context(tc.tile_pool(name="xpool", bufs=1))
        wepool = ctx.enter_context(tc.tile_pool(name="wepool", bufs=2))
        w8pool = ctx.enter_context(tc.tile_pool(name="w8pool", bufs=2))
        opool = ctx.enter_context(tc.tile_pool(name="opool", bufs=1))
        gpool = ctx.enter_context(tc.tile_pool(name="gpool", bufs=1))
        small = ctx.enter_context(tc.tile_pool(name="small", bufs=4))
        # 8 PSUM banks: 3 x y ([128,1024] f32, 2 banks each) + 2 single-bank
        # "g8" slots shared by warmup / gate logits / softmax transposes
        # (phase A), the [128,512] bias-term matmuls (phase B), and the
        # final tile's [128,256] tail groups.
        psum_y = ctx.enter_context(tc.tile_pool(name="psum_y", bufs=3, space="PSUM"))
        psum_g = ctx.enter_context(tc.tile_pool(name="psum_g", bufs=2, space="PSUM"))

        # ---- input DMAs spread across engines in NEED order ----
        # small gate constants ride gpsimd's (software) DGE; the warmup
        # tile's memset goes first (it gates the very first PE instruction),
        # then Wg - the gate matmuls start as soon as x chunk 0 lands.
        warm_sb = consts.tile([P, 512], bf16)
        nc.gpsimd.memset(warm_sb, 0.25)
        wg_sb = consts.tile([P, K_CH, P], bf16)
        nc.gpsimd.dma_start(out=wg_sb, in_=Wg[:, :, :])
        ident8 = consts.tile([P, P], fp32)
        nc.gpsimd.dma_start(out=ident8, in_=idn[:, :])
        bgc_sb = consts.tile([E, 1], fp32)
        nc.gpsimd.dma_start(out=bgc_sb, in_=bgc[:, :])
        # be and the bias-term stationary are PADDED to the full 128
        # partitions (rows 8..127 zero): a partial-height LDWEIGHTS cannot
        # use the background weight buffer (its row group conflicts with the
        # in-flight full-height matmul), costing ~2x95ns per e=1 block
        be_sb = consts.tile([P, OUT_F], bf16)
        nc.gpsimd.memset(be_sb, 0)
        nc.gpsimd.dma_start(out=be_sb[0:E, :], in_=be[:, :])

        def fetch_we_chunk(e, c, eng=None):
            s = e if e < FP8_E0 else E_BF - 1  # bf16 slot (expert 7 -> slot 5)
            wc = wepool.tile([P, OUT_F], bf16, tag=f"we{c}")
            (eng or nc.scalar).dma_start(
                out=wc, in_=We[s, c * P : (c + 1) * P, :]
            )
            return wc

        def fetch_we(e):
            return [fetch_we_chunk(e, c) for c in range(K_CH)]

        # fp8 DoubleRow tensors: x8 is one 1MB burst (8KB/partition lines),
        # W8[j] four 256KB chunk-pair tiles [p, i, o]. These tiles are fresh
        # allocations with no WAR dependency (unlike the wepool prefetches,
        # which inherit one from buffer reuse), so the scheduler would hoist
        # their DMAs into the critical head window - measured +20us from the
        # 3MB crowding out xa/We[0]. tile_wait_until pins them mid-kernel.
        x8_sb = xpool.tile([P, K_CH, n_tok_pc], f8)

        def fetch_w8(j, wait_ms):
            tiles = []
            with tc.tile_wait_until(wait_ms):
                for cp in range(K_CH // 2):
                    w = w8pool.tile([P, 2, OUT_F], f8, tag=f"w8{cp}")
                    nc.scalar.dma_start(out=w, in_=W8d[j, :, cp, :, :])
                    tiles.append(w)
            return tiles

        nh = n_tok_pc // 2
        T_half = T // 2

        # Critical-prefix DMAs in NEED order per queue: xa per-chunk on
        # sync (batching them into fewer larger DMAs measured SLOWER -
        # pair 0's first matmul waited ~2-4us longer on the merged
        # transfers); We[0] chunks 0..5 on scalar, 6..7 on sync behind xa;
        # xb chunks 0..3 on sync, 4..7 on scalar, each landing ~1us ahead
        # of pair 2/3's first use.
        xa_sb = xpool.tile([P, K_CH, nh], bf16, name="xa")
        xb_sb = xpool.tile([P, K_CH, nh], bf16, name="xb")
        we_sb = {}
        for c in range(K_CH):
            nc.sync.dma_start(out=xa_sb[:, c, :], in_=xTa[:, c, :])
        we_sb[0] = [
            fetch_we_chunk(0, c, eng=(nc.scalar if c < 6 else nc.sync))
            for c in range(K_CH)
        ]
        for c in range(K_CH):
            (nc.sync if c < 4 else nc.scalar).dma_start(
                out=xb_sb[:, c, :], in_=xTb[:, c, :]
            )

        def xslice(c, t):
            if t < T_half:
                return xa_sb[:, c, t * P : (t + 1) * P]
            return xb_sb[:, c, (t - T_half) * P : (t - T_half + 1) * P]

        # HAM warmup: the PE is idle ~3.5us waiting for the first input DMAs;
        # dummy matmuls on a memset tile during that window un-throttle the
        # clock gate (1.2 -> 2.4 GHz) so the DMA-chased pair 0 runs warm.
        wps = psum_g.tile([P, 512], fp32, tag="g8")
        for _ in range(12):
            nc.tensor.matmul(
                wps, lhsT=warm_sb[:, 0:P], rhs=warm_sb, start=True, stop=True
            )

        g_sb = gpool.tile([P, T, E], fp32)
        gTexp_sb = gpool.tile([P, T, P], bf16)
        nc.vector.memset(gTexp_sb, 0)
        gTexpF_sb = gpool.tile([P, T, P], fp32)
        nc.vector.memset(gTexpF_sb, 0)
        r_sb = gpool.tile([P, T], fp32)
        out_sb = opool.tile([P, T, OUT_F], fp32)

        def softmax(t):
            # PE-transpose the f32 exp(z+bg) back to [tok, e]; one DVE op
            # copies it out of psum and produces the softmax denominator via
            # accum_out; then normalize. (exp without max-subtraction is
            # safe: logits are O(+-3).)
            lg = psum_g.tile([P, P], fp32, tag="g8", name=f"lg{t}")
            nc.tensor.transpose(lg, gTexpF_sb[:, t, :], ident8)
            ssum = small.tile([P, 1], fp32, tag="ssum")
            nc.vector.tensor_scalar(
                g_sb[:, t, :],
                lg[:, 0:E],
                1.0,
                0.0,
                mybir.AluOpType.mult,
                mybir.AluOpType.add,
                accum_out=ssum,
            )
            nc.vector.reciprocal(out=r_sb[:, t : t + 1], in_=ssum)
            nc.vector.tensor_scalar_mul(
                g_sb[:, t, :], g_sb[:, t, :], r_sb[:, t : t + 1]
            )

        def combine0(t, py):
            # out[t] = y(e=0) * g[:, 0] - reads ONLY py, releasing the y slot
            nc.vector.tensor_scalar_mul(
                out_sb[:, t, :], py[:, :], g_sb[:, t, 0:1]
            )

        def bias_acc(t):
            # unnormalized bias term exp(z+bg) @ be in [128,512]
            # halves (the single-bank g8 psum slots), folded in normalized
            # via fused stt: out[t] = pb * r + out[t]
            for h in range(2):
                hs = slice(h * 512, (h + 1) * 512)
                pb = psum_g.tile([P, 512], fp32, tag="g8", name=f"pb{t}_{h}")
                nc.tensor.matmul(
                    pb, lhsT=gTexp_sb[:, t, :], rhs=be_sb[:, hs],
                    start=True, stop=True,
                )
                nc.vector.scalar_tensor_tensor(
                    out=out_sb[:, t, hs],
                    in0=pb[:, :],
                    scalar=r_sb[:, t : t + 1],
                    in1=out_sb[:, t, hs],
                    op0=mybir.AluOpType.mult,
                    op1=mybir.AluOpType.add,
                )

        def combine(e, t, py):
            # out[t] = y(e) * g[:, e] + out[t]   (fused on DVE)
            nc.vector.scalar_tensor_tensor(
                out=out_sb[:, t, :],
                in0=py[:, :],
                scalar=g_sb[:, t, e : e + 1],
                in1=out_sb[:, t, :],
                op0=mybir.AluOpType.mult,
                op1=mybir.AluOpType.add,
            )

        # ---- phase A: expert 0 + gate, chunk-major over tile pairs ----
        # pair p covers tiles (2p, 2p+1); the c-loop is OUTER so pair 0 needs
        # only one (xa, We[0]) chunk pair in SBUF per step. Gate half b's
        # logit matmuls ride pair b's chunk steps (they consume the same x).
        py_live = {}
        # the gate / bias / transpose stationaries are all PADDED to the
        # full 128 rows or columns (zeros) - partial-height/width LDWEIGHTS
        # conflict with in-flight full-size matmuls' row/col groups and
        # cannot use the background weight buffer (~+95ns per transition)
        lgt = [
            psum_g.tile([P, nh], fp32, tag="g8", name=f"lgt{b}") for b in range(2)
        ]

        def gate_post(b):
            # unnormalized transposed exp(z+bg) (the gate bias rides ACT's
            # per-partition bias port): f32 for the softmax transposes, bf16
            # for the bias-term matmul; frees the logits' psum bank
            bslc = slice(b * T_half, (b + 1) * T_half)
            nc.scalar.activation(
                out=gTexpF_sb[0:E, bslc, :],
                in_=lgt[b][0:E, :],
                func=mybir.ActivationFunctionType.Exp,
                bias=bgc_sb[:, :],
            )
            nc.scalar.activation(
                out=gTexp_sb[0:E, bslc, :],
                in_=lgt[b][0:E, :],
                func=mybir.ActivationFunctionType.Exp,
                bias=bgc_sb[:, :],
            )

        def pair_steps(p):
            t0, t1 = 2 * p, 2 * p + 1
            py_live[t0] = psum_y.tile([P, OUT_F], fp32, tag="y", name=f"py{t0}")
            py_live[t1] = psum_y.tile([P, OUT_F], fp32, tag="y", name=f"py{t1}")
            # gate half 0 rides pair 0 (same xa chunks); half 1 rides pair 2
            # (by then xb is fully resident - in pair 1 it would stall the
            # engine stream behind the still-streaming xb chunks)
            if p == 0:
                half = lambda c: xa_sb[:, c, :]
            elif p == 2:
                half = lambda c: xb_sb[:, c, :]
            else:
                half = None
            for c in range(K_CH):
                if half is not None:
                    nc.tensor.matmul(
                        lgt[min(p, 1)],
                        lhsT=wg_sb[:, c, :],
                        rhs=half(c),
                        start=(c == 0),
                        stop=(c == K_CH - 1),
                    )
                for t in (t0, t1):
                    for h in range(2):
                        hs = slice(h * 512, (h + 1) * 512)
                        nc.tensor.matmul(
                            py_live[t][:, hs],
                            lhsT=xslice(c, t),
                            rhs=we_sb[0][c][:, hs],
                            start=(c == 0),
                            stop=(c == K_CH - 1),
                        )
                yield c

        for c in pair_steps(0):
            pass
        gate_post(0)
        we_sb[1] = [fetch_we_chunk(1, c, eng=nc.sync) for c in range(K_CH)]

        # pairs 1..T/2-1 with softmax + combine0 sprinkled between chunk
        # steps (the Tile scheduler is dependency-driven; the sprinkle just
        # spreads the DVE/ACT load and keeps y-slot release ahead of demand)
        next_sm = 0
        next_c0 = 0
        for p in range(1, T // 2):
            sm_avail = T_half if p <= 2 else T  # gate half 1 exps after pair 2
            # pairs 1 and 3 sprinkle from step 3: their first transposes gate
            # on an exp that resolves ~1.7us after the previous pair ends, and
            # an earlier slot would stall the in-order PE stream behind it
            first_c = 3 if p in (1, T // 2 - 1) else 1
            for c in pair_steps(p):
                if c >= first_c:
                    if next_sm < sm_avail:
                        softmax(next_sm)
                        next_sm += 1
                    if next_c0 < 2 * p and next_c0 < next_sm:
                        combine0(next_c0, py_live.pop(next_c0))
                        next_c0 += 1
            if p == 2:
                gate_post(1)
        while next_sm < T:
            softmax(next_sm)
            next_sm += 1
        while next_c0 < T:
            combine0(next_c0, py_live.pop(next_c0))
            next_c0 += 1

        def main_mms(e, t):
            py = psum_y.tile([P, OUT_F], fp32, tag="y", name=f"y{e}_{t}")
            for c in range(K_CH):
                for h in range(2):
                    hs = slice(h * 512, (h + 1) * 512)
                    nc.tensor.matmul(
                        py[:, hs],
                        lhsT=xslice(c, t),
                        rhs=we_sb[e][c][:, hs],
                        start=(c == 0),
                        stop=(c == K_CH - 1),
                    )
            return py

        DR = mybir.MatmulPerfMode.DoubleRow
        NCP = K_CH // 2  # chunk pairs

        def x8slice(cp, t):
            return x8_sb[:, 2 * cp : 2 * cp + 2, t * P : (t + 1) * P]

        def main_mms8(j, t):
            # DoubleRow: each pair-matmul contracts chunks (2cp, 2cp+1) at
            # 2 fp8 MACs/cell/cycle; 8 matmuls per block instead of 16
            py = psum_y.tile([P, OUT_F], fp32, tag="y", name=f"y8{j}_{t}")
            for cp in range(NCP):
                for h in range(2):
                    hs = slice(h * 512, (h + 1) * 512)
                    nc.tensor.matmul(
                        py[:, hs],
                        lhsT=x8slice(cp, t),
                        rhs=w8_sb[j][cp][:, :, hs],
                        start=(cp == 0),
                        stop=(cp == NCP - 1),
                        perf_mode=DR,
                    )
            return py

        # ---- phase B: experts 1..4 bf16, 5..6 fp8/DoubleRow, 7 bf16 last ----
        w8_sb = {}
        for e in range(1, E):
            nxt = e + 1
            if nxt < FP8_E0:
                we_sb[nxt] = fetch_we(nxt)
                if nxt == FP8_E0 - 1:
                    # ship the fp8 x copy mid-kernel (see fetch_w8 note)
                    with tc.tile_wait_until(0.040):
                        nc.scalar.dma_start(out=x8_sb, in_=x8d[:, :, :])
            elif nxt < FP8_E0 + N_FP8:
                w8_sb[nxt - FP8_E0] = fetch_w8(
                    nxt - FP8_E0, 0.055 if nxt == FP8_E0 else 0.085
                )
            elif nxt < E:
                we_sb[nxt] = fetch_we(nxt)
            for t in range(T):
                if e == E - 1 and t == T - 1:
                    # final tile: h0 as usual (its combine + writeback overlap
                    # the rest); h1 as two [128,256] psum groups in the g8
                    # banks so only a 256-col combine + writeback trail the
                    # final matmul, issued on otherwise-idle engines
                    py = psum_y.tile([P, OUT_F], fp32, tag="y", name="ylast")
                    hs = slice(0, 512)
                    for c in range(K_CH):
                        nc.tensor.matmul(
                            py[:, hs],
                            lhsT=xslice(c, t),
                            rhs=we_sb[e][c][:, hs],
                            start=(c == 0),
                            stop=(c == K_CH - 1),
                        )
                    nc.vector.scalar_tensor_tensor(
                        out=out_sb[:, t, hs],
                        in0=py[:, hs],
                        scalar=g_sb[:, t, e : e + 1],
                        in1=out_sb[:, t, hs],
                        op0=mybir.AluOpType.mult,
                        op1=mybir.AluOpType.add,
                    )
                    nc.sync.dma_start(
                        out=out[t * P : (t + 1) * P, hs], in_=out_sb[:, t, hs]
                    )
                    for q, eng in ((2, nc.gpsimd), (3, nc.scalar)):
                        qs = slice(q * 256, (q + 1) * 256)
                        pq = psum_g.tile([P, 256], fp32, tag="g8", name=f"q{q}")
                        for c in range(K_CH):
                            nc.tensor.matmul(
                                pq,
                                lhsT=xslice(c, t),
                                rhs=we_sb[e][c][:, qs],
                                start=(c == 0),
                                stop=(c == K_CH - 1),
                            )
                        nc.vector.scalar_tensor_tensor(
                            out=out_sb[:, t, qs],
                            in0=pq[:, :],
                            scalar=g_sb[:, t, e : e + 1],
                            in1=out_sb[:, t, qs],
                            op0=mybir.AluOpType.mult,
                            op1=mybir.AluOpType.add,
                        )
                        eng.dma_start(
                            out=out[t * P : (t + 1) * P, qs],
                            in_=out_sb[:, t, qs],
                        )
                else:
                    if FP8_E0 <= e < FP8_E0 + N_FP8:
                        py = main_mms8(e - FP8_E0, t)
                    else:
                        py = main_mms(e, t)
                    combine(e, t, py)
                    if e == 1:
                        bias_acc(t)
                    if e == E - 1:
                        # write back this tile right after its final combine
                        nc.sync.dma_start(
                            out=out[t * P : (t + 1) * P, :], in_=out_sb[:, t, :]
                        )
            if e - 1 in we_sb:
                del we_sb[e - 1]

    nc.compile()
    return nc


_NC_CACHE: dict = {}


def _get_nc(n_tok_pc: int):
    if n_tok_pc not in _NC_CACHE:
        _NC_CACHE[n_tok_pc] = build_nc(n_tok_pc)
    return _NC_CACHE[n_tok_pc]


def make_in_maps(x, We, be, Wg, bg):
    """Host-side sharding: token-shard + transpose x, bf16-cast the dense
    path, e4m3-quantize the DoubleRow experts (x/8 and 8*We -- the two
    power-of-2 scales cancel exactly in the psum)."""
    bf16 = ml_dtypes.bfloat16
    f8 = ml_dtypes.float8_e4m3
    x = np.asarray(x)
    We = np.asarray(We)
    n_tok_pc = x.shape[0] // N_CORES
    We_bf = We[[0, 1, 2, 3, 4, 7]].astype(bf16)
    be_bf = np.asarray(be).astype(bf16)
    K_CH = IN_F // P
    bg_col = np.asarray(bg).astype(np.float32).reshape(E, 1)
    # [1024, 8] -> [p, chunk, e], padded to 128 "experts" (zeros)
    Wg_pad = np.zeros((P, K_CH, P), dtype=bf16)
    Wg_pad[:, :, :E] = (
        np.asarray(Wg).astype(bf16).reshape(K_CH, P, E).transpose(1, 0, 2)
    )
    ident = np.eye(P, dtype=np.float32)
    xbf = x.astype(bf16)
    # W8[j, p, cp, i, o] = e4m3(8*We[FP8_E0+j, (2cp+i)*128+p, o])
    W8 = np.ascontiguousarray(
        (8.0 * We[FP8_E0 : FP8_E0 + N_FP8])
        .astype(f8)
        .reshape(N_FP8, K_CH // 2, 2, P, OUT_F)
        .transpose(0, 3, 1, 2, 4)
    )
    nh = n_tok_pc // 2
    in_maps = []
    for cid in range(N_CORES):
        sl = slice(cid * n_tok_pc, (cid + 1) * n_tok_pc)
        xs = xbf[sl]
        xsT = xs.T
        # xTa/xTb[p, c, n] = x[(half)+n, c*128+p] (token halves, SBUF layout)
        xTa = np.ascontiguousarray(
            xsT[:, :nh].reshape(K_CH, P, nh).transpose(1, 0, 2)
        )
        xTb = np.ascontiguousarray(
            xsT[:, nh:].reshape(K_CH, P, nh).transpose(1, 0, 2)
        )
        # x8[p, c, n] = e4m3(x[n, c*128+p]/8)
        x8 = np.ascontiguousarray(
            (x[sl] / 8.0)
            .astype(f8)
            .T.reshape(K_CH, P, n_tok_pc)
            .transpose(1, 0, 2)
        )
        in_maps.append(
            {
                "xTa": xTa,
                "xTb": xTb,
                "We": We_bf,
                "x8": x8,
                "W8": W8,
                "be": be_bf,
                "Wg": Wg_pad,
                "bgc": bg_col,
                "idn": ident,
            }
        )
    return in_maps, n_tok_pc


def run(x, We, be, Wg, bg, trace=False, **trace_kwargs):
    in_maps, n_tok_pc = make_in_maps(x, We, be, Wg, bg)
    nc = _get_nc(n_tok_pc)
    res = run_bass_kernel_spmd(
        nc, in_maps, core_ids=list(range(N_CORES)), trace=trace, **trace_kwargs
    )
    outs = [res.results[i]["out"] for i in range(N_CORES)]
    return np.concatenate(outs, axis=0).astype(np.float32), res


def kernel(x, We, be, Wg, bg):
    out, _ = run(x, We, be, Wg, bg, trace=False)
    return out



# revision 38
# speedup vs baseline: 1.0078x; 1.0078x over previous
"""Dense MoE layer (8 experts, all-expert weighted combine) on 8 TRN2 NeuronCores.

Strategy: data-parallel over the token dim. Each core gets a 1024-token shard
(pre-transposed + bf16-cast on host), the full stacked expert weights (bf16),
and computes gate softmax + all 8 expert matmuls + gate-weighted combine
locally. No collectives; host concatenates the 8 output shards.

The kernel is PE-streaming-bound: 1024 expert matmuls of [128x128]@[128x512]
at the 216 ns bf16 floor. The schedule minimizes everything around that
stream:

  - phase A (expert 0 + gate) runs CHUNK-major over tile-PAIRS so the PE
    starts on real work as soon as the first x / We[0] chunks land instead of
    waiting for the full critical prefix. x ships in half-token chunks: pair 0
    (tiles 0,1) + the first gate half need only xa + We[0] = 3MB before
    full-rate compute; xb streams in behind them and pair 2's steps carry the
    second gate half's matmuls. 12 dummy matmuls on a memset tile warm the
    HAM clock gate through the DMA lead-in, handing off seamlessly so the
    DMA-chased pair 0 runs at 2.4 GHz.
  - gate logits are computed TRANSPOSED (lhsT = Wg chunk, 8-column weight
    loads are ~free, padded to 128 cols) into [128,512] psum per half. exp(z+bg) goes straight from
    psum to [8,tok] SBUF tensors on ACT (f32 for the softmax transposes, bf16
    for the bias matmul) - the gate bias rides ACT's per-partition bias port,
    so there is no separate bias add and no logit copy. Per tile, softmax is:
    PE-transpose the f32 exp back to [128,8], one DVE copy-with-accum (the
    denominator), reciprocal, scale. The bias term exp(z+bg) @ be is deferred
    to the e=1 sweep (phase B) so phase A can hold THREE [128,1024] psum
    y-slots (6 banks) + 2 shared single-bank "g8" slots.
  - phase B (experts 1..7) is token-major with 16 matmuls per (e,t) block
    accumulating in psum f32; combine is one fused DVE op:
    out = psum*g[:,e] + out. Expert weights double-buffer through SBUF.
  - DMA queues move ~one 256KB chunk per 1.3us each, so the critical
    prefix is split need-ordered across both hardware queues: sync = xa,
    then We[1], then xb, then outputs; scalar = We[0], then per-expert
    prefetch; gpsimd = small consts. xb/We[1] are emitted after gate_post(0)
    and ride sync so their dma_start issues never sit in front of the
    pair-0-gated exps on scalar (that chain gates the first softmax
    transpose).
  - tail: the last (e,t) block computes its second half as two [128,256]
    psum groups in the (by then free) g8 banks, so only a 256-col combine +
    writeback trails the final matmul, on otherwise-idle engines.
  - experts 5..6 run in fp8 e4m3 with perf_mode=DoubleRow: the PE packs two
    fp8 weights per cell, so each [256x128]@[256x512] pair-matmul covers two
    128-chunks of the contraction in ~one matmul time (2x ALU; measured
    ~216ns/pair-MM = 1.79x per block). x ships a second copy quantized as
    e4m3(x/8) (stationary) and We[5..6] as e4m3(8*We) (moving); the two
    power-of-2 scales cancel exactly, so the combine path is unchanged.
    Quantization noise on 2 of 8 experts puts the end-to-end rel err at
    ~1.8e-2 vs the 2e-2 gate (bf16 path alone: 2.5e-3). Expert 7 stays bf16
    and runs LAST: its 27.6us sweep hides the ~21us of per-tile output
    writebacks that a 13.7us fp8 sweep cannot (measured: fp8-last stalls
    5.9us at the tail). The fp8 tensors stream mid-phase-B on the scalar
    queue (x8 at the e=3 sweep, W8[j] one sweep ahead of use), far off the
    critical prefix.
"""

import os
import sys

import numpy as np

try:
    import concourse.bass as bass  # noqa: F401
except ImportError:  # harness containers stage the repo at /opt/trn_rl_repo
    sys.path.insert(0, "/opt/trn_rl_repo")

from contextlib import ExitStack

import ml_dtypes

import concourse.bass as bass
import concourse.mybir as mybir
import concourse.tile as tile
from concourse import bacc
from concourse.bass_utils import run_bass_kernel_spmd

N_CORES = 8
N_TOK = 8192
IN_F = 1024
OUT_F = 1024
E = 8
P = 128  # partitions
N_FP8 = 2  # experts 5..6 run fp8/DoubleRow; 0..4 and 7 stay bf16
FP8_E0 = 5  # first fp8 expert
E_BF = E - N_FP8  # bf16 expert count (slots 0..4 = experts 0..4, slot 5 = expert 7)


def build_nc(n_tok_pc: int = N_TOK // N_CORES, debug: bool = False):
    """Build the single-core SPMD Bass program (same program on all 8 cores)."""
    fp32 = mybir.dt.float32
    bf16 = mybir.dt.bfloat16
    f8 = mybir.dt.float8e4

    K_CH = IN_F // P  # contraction chunks of 128
    T = n_tok_pc // P  # token tiles per core
    assert T >= 4 and T % 2 == 0

    nc = bacc.Bacc(
        "TRN2", target_bir_lowering=False, debug=debug, enable_asserts=False
    )

    # both token halves in SBUF layout [p, chunk, tok] so they ship in few
    # large DMAs (each dma_start costs ~600ns of engine issue time, and 16
    # per-chunk transfers rotate through too few DMA semaphores - the false
    # coupling made pair-0 matmuls wait on unrelated xb chunks)
    xTa = nc.declare_dram_parameter(
        "xTa", [P, IN_F // P, n_tok_pc // 2], bf16, isOutput=False
    )
    xTb = nc.declare_dram_parameter(
        "xTb", [P, IN_F // P, n_tok_pc // 2], bf16, isOutput=False
    )
    We = nc.declare_dram_parameter("We", [E_BF, IN_F, OUT_F], bf16, isOutput=False)
    # fp8 copies for the DoubleRow experts: x8[p, c, n] = e4m3(x[n, c*128+p]/8),
    # W8[j, p, cp, i, o] = e4m3(8*We[FP8_E0+j, (2cp+i)*128+p, o])
    x8d = nc.declare_dram_parameter("x8", [P, K_CH, n_tok_pc], f8, isOutput=False)
    W8d = nc.declare_dram_parameter(
        "W8", [N_FP8, P, K_CH // 2, 2, OUT_F], f8, isOutput=False
    )
    be = nc.declare_dram_parameter("be", [E, OUT_F], bf16, isOutput=False)
    Wg = nc.declare_dram_parameter("Wg", [P, K_CH, P], bf16, isOutput=False)
    bgc = nc.declare_dram_parameter("bgc", [E, 1], fp32, isOutput=False)
    idn = nc.declare_dram_parameter("idn", [P, P], fp32, isOutput=False)
    out = nc.declare_dram_parameter("out", [n_tok_pc, OUT_F], fp32, isOutput=True)

    with tile.TileContext(nc) as tc, ExitStack() as ctx:
        consts = ctx.enter_context(tc.tile_pool(name="consts", bufs=1))
        xpool = ctx.enter_context(tc.tile_pool(name="xpool", bufs=1))
        wepool = ctx.enter_context(tc.tile_pool(name="wepool", bufs=2))
        w8pool = ctx.enter_context(tc.tile_pool(name="w8pool", bufs=2))
        opool = ctx.enter_context(tc.tile_pool(name="opool", bufs=1))
        gpool = ctx.enter_context(tc.tile_pool(name="gpool", bufs=1))
        small = ctx.enter_context(tc.tile_pool(name="small", bufs=4))
        # 8 PSUM banks: 3 x y ([128,1024] f32, 2 banks each) + 2 single-bank
        # "g8" slots shared by warmup / gate logits / softmax transposes
        # (phase A), the [128,512] bias-term matmuls (phase B), and the
        # final tile's [128,256] tail groups.
        psum_y = ctx.enter_context(tc.tile_pool(name="psum_y", bufs=3, space="PSUM"))
        psum_g = ctx.enter_context(tc.tile_pool(name="psum_g", bufs=2, space="PSUM"))

        # ---- input DMAs spread across engines in NEED order ----
        # small gate constants ride gpsimd's (software) DGE; the warmup
        # tile's memset goes first (it gates the very first PE instruction),
        # then Wg - the gate matmuls start as soon as x chunk 0 lands.
        warm_sb = consts.tile([P, 512], bf16)
        nc.gpsimd.memset(warm_sb, 0.25)
        wg_sb = consts.tile([P, K_CH, P], bf16)
        nc.gpsimd.dma_start(out=wg_sb, in_=Wg[:, :, :])
        ident8 = consts.tile([P, P], fp32)
        nc.gpsimd.dma_start(out=ident8, in_=idn[:, :])
        bgc_sb = consts.tile([E, 1], fp32)
        nc.gpsimd.dma_start(out=bgc_sb, in_=bgc[:, :])
        # be and the bias-term stationary are PADDED to the full 128
        # partitions (rows 8..127 zero): a partial-height LDWEIGHTS cannot
        # use the background weight buffer (its row group conflicts with the
        # in-flight full-height matmul), costing ~2x95ns per e=1 block
        be_sb = consts.tile([P, OUT_F], bf16)
        nc.gpsimd.memset(be_sb, 0)
        nc.gpsimd.dma_start(out=be_sb[0:E, :], in_=be[:, :])

        def fetch_we_chunk(e, c, eng=None):
            s = e if e < FP8_E0 else E_BF - 1  # bf16 slot (expert 7 -> slot 5)
            wc = wepool.tile([P, OUT_F], bf16, tag=f"we{c}")
            (eng or nc.scalar).dma_start(
                out=wc, in_=We[s, c * P : (c + 1) * P, :]
            )
            return wc

        def fetch_we(e):
            return [fetch_we_chunk(e, c) for c in range(K_CH)]

        # fp8 DoubleRow tensors: x8 is one 1MB burst (8KB/partition lines),
        # W8[j] four 256KB chunk-pair tiles [p, i, o]. These tiles are fresh
        # allocations with no WAR dependency (unlike the wepool prefetches,
        # which inherit one from buffer reuse), so the scheduler would hoist
        # their DMAs into the critical head window - measured +20us from the
        # 3MB crowding out xa/We[0]. tile_wait_until pins them mid-kernel.
        x8_sb = xpool.tile([P, K_CH, n_tok_pc], f8)

        def fetch_w8(j, wait_ms):
            tiles = []
            with tc.tile_wait_until(wait_ms):
                for cp in range(K_CH // 2):
                    w = w8pool.tile([P, 2, OUT_F], f8, tag=f"w8{cp}")
                    nc.scalar.dma_start(out=w, in_=W8d[j, :, cp, :, :])
                    tiles.append(w)
            return tiles

        nh = n_tok_pc // 2
        T_half = T // 2

        # Critical-prefix DMAs in NEED order per queue: xa per-chunk on
        # sync (batching them into fewer larger DMAs measured SLOWER -
        # pair 0's first matmul waited ~2-4us longer on the merged
        # transfers); We[0] chunks 0..5 on scalar, 6..7 on sync behind xa;
        # xb chunks 0..3 on sync, 4..7 on scalar, each landing ~1us ahead
        # of pair 2/3's first use.
        xa_sb = xpool.tile([P, K_CH, nh], bf16, name="xa")
        xb_sb = xpool.tile([P, K_CH, nh], bf16, name="xb")
        we_sb = {}
        for c in range(K_CH):
            nc.sync.dma_start(out=xa_sb[:, c, :], in_=xTa[:, c, :])
        we_sb[0] = [
            fetch_we_chunk(0, c, eng=(nc.scalar if c < 6 else nc.sync))
            for c in range(K_CH)
        ]
        for c in range(K_CH):
            (nc.sync if c < 4 else nc.scalar).dma_start(
                out=xb_sb[:, c, :], in_=xTb[:, c, :]
            )

        def xslice(c, t):
            if t < T_half:
                return xa_sb[:, c, t * P : (t + 1) * P]
            return xb_sb[:, c, (t - T_half) * P : (t - T_half + 1) * P]

        # HAM warmup: the PE is idle ~3.5us waiting for the first input DMAs;
        # dummy matmuls on a memset tile during that window un-throttle the
        # clock gate (1.2 -> 2.4 GHz) so the DMA-chased pair 0 runs warm.
        wps = psum_g.tile([P, 512], fp32, tag="g8")
        for _ in range(12):
            nc.tensor.matmul(
                wps, lhsT=warm_sb[:, 0:P], rhs=warm_sb, start=True, stop=True
            )

        g_sb = gpool.tile([P, T, E], fp32)
        gTexp_sb = gpool.tile([P, T, P], bf16)
        nc.vector.memset(gTexp_sb, 0)
        gTexpF_sb = gpool.tile([P, T, P], fp32)
        nc.vector.memset(gTexpF_sb, 0)
        r_sb = gpool.tile([P, T], fp32)
        out_sb = opool.tile([P, T, OUT_F], fp32)

        def softmax(t):
            # PE-transpose the f32 exp(z+bg) back to [tok, e]; one DVE op
            # copies it out of psum and produces the softmax denominator via
            # accum_out; then normalize. (exp without max-subtraction is
            # safe: logits are O(+-3).)
            lg = psum_g.tile([P, P], fp32, tag="g8", name=f"lg{t}")
            nc.tensor.transpose(lg, gTexpF_sb[:, t, :], ident8)
            ssum = small.tile([P, 1], fp32, tag="ssum")
            nc.vector.tensor_scalar(
                g_sb[:, t, :],
                lg[:, 0:E],
                1.0,
                0.0,
                mybir.AluOpType.mult,
                mybir.AluOpType.add,
                accum_out=ssum,
            )
            nc.vector.reciprocal(out=r_sb[:, t : t + 1], in_=ssum)
            nc.vector.tensor_scalar_mul(
                g_sb[:, t, :], g_sb[:, t, :], r_sb[:, t : t + 1]
            )

        def combine0(t, py):
            # out[t] = y(e=0) * g[:, 0] - reads ONLY py, releasing the y slot
            nc.vector.tensor_scalar_mul(
                out_sb[:, t, :], py[:, :], g_sb[:, t, 0:1]
            )

        def bias_acc(t):
            # unnormalized bias term exp(z+bg) @ be in [128,512]
            # halves (the single-bank g8 psum slots), folded in normalized
            # via fused stt: out[t] = pb * r + out[t]
            for h in range(2):
                hs = slice(h * 512, (h + 1) * 512)
                pb = psum_g.tile([P, 512], fp32, tag="g8", name=f"pb{t}_{h}")
                nc.tensor.matmul(
                    pb, lhsT=gTexp_sb[:, t, :], rhs=be_sb[:, hs],
                    start=True, stop=True,
                )
                nc.vector.scalar_tensor_tensor(
                    out=out_sb[:, t, hs],
                    in0=pb[:, :],
                    scalar=r_sb[:, t : t + 1],
                    in1=out_sb[:, t, hs],
                    op0=mybir.AluOpType.mult,
                    op1=mybir.AluOpType.add,
                )

        def combine(e, t, py):
            # out[t] = y(e) * g[:, e] + out[t]   (fused on DVE)
            nc.vector.scalar_tensor_tensor(
                out=out_sb[:, t, :],
                in0=py[:, :],
                scalar=g_sb[:, t, e : e + 1],
                in1=out_sb[:, t, :],
                op0=mybir.AluOpType.mult,
                op1=mybir.AluOpType.add,
            )

        # ---- phase A: expert 0 + gate, chunk-major over tile pairs ----
        # pair p covers tiles (2p, 2p+1); the c-loop is OUTER so pair 0 needs
        # only one (xa, We[0]) chunk pair in SBUF per step. Gate half b's
        # logit matmuls ride pair b's chunk steps (they consume the same x).
        py_live = {}
        # the gate / bias / transpose stationaries are all PADDED to the
        # full 128 rows or columns (zeros) - partial-height/width LDWEIGHTS
        # conflict with in-flight full-size matmuls' row/col groups and
        # cannot use the background weight buffer (~+95ns per transition)
        lgt = [
            psum_g.tile([P, nh], fp32, tag="g8", name=f"lgt{b}") for b in range(2)
        ]

        def gate_post(b):
            # unnormalized transposed exp(z+bg) (the gate bias rides ACT's
            # per-partition bias port): f32 for the softmax transposes, bf16
            # for the bias-term matmul; frees the logits' psum bank
            bslc = slice(b * T_half, (b + 1) * T_half)
            nc.scalar.activation(
                out=gTexpF_sb[0:E, bslc, :],
                in_=lgt[b][0:E, :],
                func=mybir.ActivationFunctionType.Exp,
                bias=bgc_sb[:, :],
            )
            nc.scalar.activation(
                out=gTexp_sb[0:E, bslc, :],
                in_=lgt[b][0:E, :],
                func=mybir.ActivationFunctionType.Exp,
                bias=bgc_sb[:, :],
            )

        def pair_steps(p):
            t0, t1 = 2 * p, 2 * p + 1
            py_live[t0] = psum_y.tile([P, OUT_F], fp32, tag="y", name=f"py{t0}")
            py_live[t1] = psum_y.tile([P, OUT_F], fp32, tag="y", name=f"py{t1}")
            # gate half 0 rides pair 0 (same xa chunks); half 1 rides pair 2
            # (by then xb is fully resident - in pair 1 it would stall the
            # engine stream behind the still-streaming xb chunks)
            if p == 0:
                half = lambda c: xa_sb[:, c, :]
            elif p == 2:
                half = lambda c: xb_sb[:, c, :]
            else:
                half = None
            for c in range(K_CH):
                if half is not None:
                    nc.tensor.matmul(
                        lgt[min(p, 1)],
                        lhsT=wg_sb[:, c, :],
                        rhs=half(c),
                        start=(c == 0),
                        stop=(c == K_CH - 1),
                    )
                for t in (t0, t1):
                    for h in range(2):
                        hs = slice(h * 512, (h + 1) * 512)
                        nc.tensor.matmul(
                            py_live[t][:, hs],
                            lhsT=xslice(c, t),
                            rhs=we_sb[0][c][:, hs],
                            start=(c == 0),
                            stop=(c == K_CH - 1),
                        )
                yield c

        for c in pair_steps(0):
            pass
        gate_post(0)
        we_sb[1] = [fetch_we_chunk(1, c, eng=nc.sync) for c in range(K_CH)]

        # pairs 1..T/2-1 with softmax + combine0 sprinkled between chunk
        # steps (the Tile scheduler is dependency-driven; the sprinkle just
        # spreads the DVE/ACT load and keeps y-slot release ahead of demand)
        next_sm = 0
        next_c0 = 0
        for p in range(1, T // 2):
            sm_avail = T_half if p <= 2 else T  # gate half 1 exps after pair 2
            # pairs 1 and 3 sprinkle from step 3: their first transposes gate
            # on an exp that resolves ~1.7us after the previous pair ends, and
            # an earlier slot would stall the in-order PE stream behind it
            first_c = 3 if p in (1, T // 2 - 1) else 1
            for c in pair_steps(p):
                if c >= first_c:
                    if next_sm < sm_avail:
                        softmax(next_sm)
                        next_sm += 1
                    if next_c0 < 2 * p and next_c0 < next_sm:
                        combine0(next_c0, py_live.pop(next_c0))
                        next_c0 += 1
            if p == 2:
                gate_post(1)
        while next_sm < T:
            softmax(next_sm)
            next_sm += 1
        while next_c0 < T:
            combine0(next_c0, py_live.pop(next_c0))
            next_c0 += 1

        def main_mms(e, t):
            py = psum_y.tile([P, OUT_F], fp32, tag="y", name=f"y{e}_{t}")
            for c in range(K_CH):
                for h in range(2):
                    hs = slice(h * 512, (h + 1) * 512)
                    nc.tensor.matmul(
                        py[:, hs],
                        lhsT=xslice(c, t),
                        rhs=we_sb[e][c][:, hs],
                        start=(c == 0),
                        stop=(c == K_CH - 1),
                    )
            return py

        DR = mybir.MatmulPerfMode.DoubleRow
        NCP = K_CH // 2  # chunk pairs

        def x8slice(cp, t):
            return x8_sb[:, 2 * cp : 2 * cp + 2, t * P : (t + 1) * P]

        def main_mms8(j, t):
            # DoubleRow: each pair-matmul contracts chunks (2cp, 2cp+1) at
            # 2 fp8 MACs/cell/cycle; 8 matmuls per block instead of 16
            py = psum_y.tile([P, OUT_F], fp32, tag="y", name=f"y8{j}_{t}")
            for cp in range(NCP):
                for h in range(2):
                    hs = slice(h * 512, (h + 1) * 512)
                    nc.tensor.matmul(
                        py[:, hs],
                        lhsT=x8slice(cp, t),
                        rhs=w8_sb[j][cp][:, :, hs],
                        start=(cp == 0),
                        stop=(cp == NCP - 1),
                        perf_mode=DR,
                    )
            return py

        # ---- phase B: experts 1..4 bf16, 5..6 fp8/DoubleRow, 7 bf16 last ----
        w8_sb = {}
        for e in range(1, E):
            nxt = e + 1
            if nxt < FP8_E0:
                we_sb[nxt] = fetch_we(nxt)
                if nxt == FP8_E0 - 1:
                    # ship the fp8 x copy mid-kernel (see fetch_w8 note)
                    with tc.tile_wait_until(0.040):
                        nc.scalar.dma_start(out=x8_sb, in_=x8d[:, :, :])
            elif nxt < FP8_E0 + N_FP8:
                w8_sb[nxt - FP8_E0] = fetch_w8(
                    nxt - FP8_E0, 0.055 if nxt == FP8_E0 else 0.085
                )
            elif nxt < E:
                we_sb[nxt] = fetch_we(nxt)
            for t in range(T):
                if e == E - 1 and t == T - 1:
                    # final tile: h0 as usual (its combine + writeback overlap
                    # the rest); h1 as two [128,256] psum groups in the g8
                    # banks so only a 256-col combine + writeback trail the
                    # final matmul, issued on otherwise-idle engines
                    py = psum_y.tile([P, OUT_F], fp32, tag="y", name="ylast")
                    hs = slice(0, 512)
                    for c in range(K_CH):
                        nc.tensor.matmul(
                            py[:, hs],
                            lhsT=xslice(c, t),
                            rhs=we_sb[e][c][:, hs],
                            start=(c == 0),
                            stop=(c == K_CH - 1),
                        )
                    nc.vector.scalar_tensor_tensor(
                        out=out_sb[:, t, hs],
                        in0=py[:, hs],
                        scalar=g_sb[:, t, e : e + 1],
                        in1=out_sb[:, t, hs],
                        op0=mybir.AluOpType.mult,
                        op1=mybir.AluOpType.add,
                    )
                    nc.sync.dma_start(
                        out=out[t * P : (t + 1) * P, hs], in_=out_sb[:, t, hs]
                    )
                    # q2 rides sync (behind h0): gpsimd's software queue
                    # drains slowly and its last transfer gates the epilogue
                    for q, eng in ((2, nc.sync), (3, nc.scalar)):
                        qs = slice(q * 256, (q + 1) * 256)
                        pq = psum_g.tile([P, 256], fp32, tag="g8", name=f"q{q}")
                        for c in range(K_CH):
                            nc.tensor.matmul(
                                pq,
                                lhsT=xslice(c, t),
                                rhs=we_sb[e][c][:, qs],
                                start=(c == 0),
                                stop=(c == K_CH - 1),
                            )
                        nc.vector.scalar_tensor_tensor(
                            out=out_sb[:, t, qs],
                            in0=pq[:, :],
                            scalar=g_sb[:, t, e : e + 1],
                            in1=out_sb[:, t, qs],
                            op0=mybir.AluOpType.mult,
                            op1=mybir.AluOpType.add,
                        )
                        eng.dma_start(
                            out=out[t * P : (t + 1) * P, qs],
                            in_=out_sb[:, t, qs],
                        )
                else:
                    if FP8_E0 <= e < FP8_E0 + N_FP8:
                        py = main_mms8(e - FP8_E0, t)
                    else:
                        py = main_mms(e, t)
                    combine(e, t, py)
                    if e == 1:
                        bias_acc(t)
                    if e == E - 1:
                        # write back this tile right after its final combine
                        nc.sync.dma_start(
                            out=out[t * P : (t + 1) * P, :], in_=out_sb[:, t, :]
                        )
            if e - 1 in we_sb:
                del we_sb[e - 1]

    nc.compile()
    return nc


_NC_CACHE: dict = {}


def _get_nc(n_tok_pc: int):
    if n_tok_pc not in _NC_CACHE:
        _NC_CACHE[n_tok_pc] = build_nc(n_tok_pc)
    return _NC_CACHE[n_tok_pc]


def make_in_maps(x, We, be, Wg, bg):
    """Host-side sharding: token-shard + transpose x, bf16-cast the dense
    path, e4m3-quantize the DoubleRow experts (x/8 and 8*We -- the two
    power-of-2 scales cancel exactly in the psum)."""
    bf16 = ml_dtypes.bfloat16
    f8 = ml_dtypes.float8_e4m3
    x = np.asarray(x)
    We = np.asarray(We)
    n_tok_pc = x.shape[0] // N_CORES
    We_bf = We[[0, 1, 2, 3, 4, 7]].astype(bf16)
    be_bf = np.asarray(be).astype(bf16)
    K_CH = IN_F // P
    bg_col = np.asarray(bg).astype(np.float32).reshape(E, 1)
    # [1024, 8] -> [p, chunk, e], padded to 128 "experts" (zeros)
    Wg_pad = np.zeros((P, K_CH, P), dtype=bf16)
    Wg_pad[:, :, :E] = (
        np.asarray(Wg).astype(bf16).reshape(K_CH, P, E).transpose(1, 0, 2)
    )
    ident = np.eye(P, dtype=np.float32)
    xbf = x.astype(bf16)
    # W8[j, p, cp, i, o] = e4m3(8*We[FP8_E0+j, (2cp+i)*128+p, o])
    W8 = np.ascontiguousarray(
        (8.0 * We[FP8_E0 : FP8_E0 + N_FP8])
        .astype(f8)
        .reshape(N_FP8, K_CH // 2, 2, P, OUT_F)
        .transpose(0, 3, 1, 2, 4)
    )
    nh = n_tok_pc // 2
    in_maps = []
    for cid in range(N_CORES):
        sl = slice(cid * n_tok_pc, (cid + 1) * n_tok_pc)
        xs = xbf[sl]
        xsT = xs.T
        # xTa/xTb[p, c, n] = x[(half)+n, c*128+p] (token halves, SBUF layout)
        xTa = np.ascontiguousarray(
            xsT[:, :nh].reshape(K_CH, P, nh).transpose(1, 0, 2)
        )
        xTb = np.ascontiguousarray(
            xsT[:, nh:].reshape(K_CH, P, nh).transpose(1, 0, 2)
        )
        # x8[p, c, n] = e4m3(x[n, c*128+p]/8)
        x8 = np.ascontiguousarray(
            (x[sl] / 8.0)
            .astype(f8)
            .T.reshape(K_CH, P, n_tok_pc)
            .transpose(1, 0, 2)
        )
        in_maps.append(
            {
                "xTa": xTa,
                "xTb": xTb,
                "We": We_bf,
                "x8": x8,
                "W8": W8,
                "be": be_bf,
                "Wg": Wg_pad,
                "bgc": bg_col,
                "idn": ident,
            }
        )
    return in_maps, n_tok_pc


def run(x, We, be, Wg, bg, trace=False, **trace_kwargs):
    in_maps, n_tok_pc = make_in_maps(x, We, be, Wg, bg)
    nc = _get_nc(n_tok_pc)
    res = run_bass_kernel_spmd(
        nc, in_maps, core_ids=list(range(N_CORES)), trace=trace, **trace_kwargs
    )
    outs = [res.results[i]["out"] for i in range(N_CORES)]
    return np.concatenate(outs, axis=0).astype(np.float32), res


def kernel(x, We, be, Wg, bg):
    out, _ = run(x, We, be, Wg, bg, trace=False)
    return out

